# revision 7
# baseline (speedup 1.0000x reference)
"""DREAMReconstructor Trainium2 kernel.

Strategy: data-parallel over batch across 8 NeuronCores (8 rows/core).
Per core, a fully-unrolled 256-step recurrence where every matmul keeps
batch (M=8) as the PE stationary-column dim:

  - one combined weight matrix W_all = [W_rec; W_pred; W_dec] streamed as
    the moving operand, 4-way column-tiled across PE quadrants so the four
    K-chunks' waves run concurrently (out rows 32j hold h@W_rec (pre),
    h@W_pred (xhat), h@W_dec (recon)),
  - x@W_in.T precomputed for all t (xin), accumulated into the pre psum
    via identity-rhs transpose matmuls (no extra vector add),
  - -x_t accumulated into the xhat psum the same way, so the error
    reduction is a single ACT Square with accum_out,
  - low-rank fast weights A [(b,r)=128, h=512] live in SBUF; readout is a
    masked-selector matmul accumulated straight into pre; the Hebbian
    outer-product update is one K=8 matmul into psum plus a single
    fused scalar_tensor_tensor decay+add,
  - the leaky h update h' = tanh(pre)*iv + h*(1-iv) is computed by the PE
    via a stacked [40,512] stationary against a per-step diagonal matrix
    built with one fused vector op.
"""
import sys
import numpy as np

sys.path.insert(0, "/opt/trn_rl_repo")

import concourse.bass as bass
import concourse.tile as tile
from concourse import bacc, mybir
from concourse.bass_utils import run_bass_kernel_spmd

F32 = mybir.dt.float32

B, T, D, H, R = 64, 256, 256, 512, 16
NCORES = 8
BL = B // NCORES  # 8 rows per core

FORGET = 0.005
BASE_PLAST = 0.5
BASE_THRESH = 0.3
SURP_TEMP = 0.05
ERR_SMOOTH = 0.05
LTC_TAU = 5.0
LTC_SCALE = 5.0
KAPPA = 0.5
SLEEP_RATE = 0.01
MIN_SURP = 0.15

_CACHE = {}


def _host_constants(W_in, W_rec, W_pred, B_proj, W_dec, b_dec):
    c = {}
    W_all = np.concatenate([W_rec, W_pred, W_dec], axis=0)          # [1024, 512]
    c["W_allT"] = np.ascontiguousarray(W_all.T.reshape(4, 128, 1024, order="F")
                                       if False else
                                       W_all.T.reshape(512, 1024).reshape(4, 128, 1024))
    c["W_inT"] = np.ascontiguousarray(W_in.T.reshape(2, 128, H))     # [256,512]->[2,128,512]
    c["BprojT"] = np.ascontiguousarray(B_proj.T.reshape(2, 128, R))  # [256,16]->[2,128,16]
    c["I128"] = np.eye(128, dtype=np.float32)
    c["nI128"] = -np.eye(128, dtype=np.float32)
    e = np.zeros((128, 8), np.float32)
    for p in range(128):
        e[p, p // 16] = KAPPA                                        # kappa folded into readout
    c["E_half"] = e
    c["E_bT"] = np.ascontiguousarray((e.T != 0).astype(np.float32))  # [8,128] 0/1
    em = np.zeros((128, 8), np.float32)
    for j in range(8):
        em[64 + j, j] = 1.0
    c["E_move"] = em
    es = np.zeros((40, 8), np.float32)
    eo = np.zeros((40, 8), np.float32)
    for b in range(8):
        es[b, b] = -1.0          # h_b side: om = 0.8 - surp
        eo[b, b] = 0.8
        es[32 + b, b] = 1.0      # th side: iv = 0.2 + surp
        eo[32 + b, b] = 0.2
    c["Esign40"] = es
    c["Eoff40"] = eo
    c["ones18"] = np.ones((1, 8), np.float32)
    c["bdec_row"] = np.ascontiguousarray(b_dec.reshape(1, D))
    return c


def _build(nsteps):
    nc = bacc.Bacc("TRN2", target_bir_lowering=False, debug=False, num_devices=NCORES)

    d_xT = nc.dram_tensor("xT", [128, 2, T, BL], F32, kind="ExternalInput")
    d_WallT = nc.dram_tensor("W_allT", [4, 128, 1024], F32, kind="ExternalInput")
    d_WinT = nc.dram_tensor("W_inT", [2, 128, H], F32, kind="ExternalInput")
    d_BpT = nc.dram_tensor("BprojT", [2, 128, R], F32, kind="ExternalInput")
    d_I = nc.dram_tensor("I128", [128, 128], F32, kind="ExternalInput")
    d_nI = nc.dram_tensor("nI128", [128, 128], F32, kind="ExternalInput")
    d_Eh = nc.dram_tensor("E_half", [128, 8], F32, kind="ExternalInput")
    d_EbT = nc.dram_tensor("E_bT", [8, 128], F32, kind="ExternalInput")
    d_Emv = nc.dram_tensor("E_move", [128, 8], F32, kind="ExternalInput")
    d_Es = nc.dram_tensor("Esign40", [40, 8], F32, kind="ExternalInput")
    d_Eo = nc.dram_tensor("Eoff40", [40, 8], F32, kind="ExternalInput")
    d_o18 = nc.dram_tensor("ones18", [1, 8], F32, kind="ExternalInput")
    d_bd = nc.dram_tensor("bdec_row", [1, D], F32, kind="ExternalInput")
    d_y = nc.dram_tensor("y", [BL, T, D], F32, kind="ExternalOutput")

    AL = mybir.AluOpType
    AF = mybir.ActivationFunctionType

    with tile.TileContext(nc) as tc:
        with tc.tile_pool(name="persist", bufs=1) as P:
            # persistent tiles
            WallT = [P.tile([128, 1024], F32, tag=f"WallT{i}", name=f"WallT{i}") for i in range(4)]
            for kc in range(4):
                nc.sync.dma_start(WallT[kc][:], d_WallT[kc])
            I128 = P.tile([128, 128], F32); nc.sync.dma_start(I128[:], d_I[:])
            nI128 = P.tile([128, 128], F32); nc.sync.dma_start(nI128[:], d_nI[:])
            E_half = P.tile([128, 8], F32); nc.sync.dma_start(E_half[:], d_Eh[:])
            E_bT = P.tile([8, 128], F32); nc.sync.dma_start(E_bT[:], d_EbT[:])
            E_move = P.tile([128, 8], F32); nc.sync.dma_start(E_move[:], d_Emv[:])
            Es40 = P.tile([40, 8], F32); nc.sync.dma_start(Es40[:], d_Es[:])
            Eo40 = P.tile([40, 8], F32); nc.sync.dma_start(Eo40[:], d_Eo[:])
            o18 = P.tile([1, 8], F32); nc.sync.dma_start(o18[:], d_o18[:])
            bdec = P.tile([1, D], F32); nc.sync.dma_start(bdec[:], d_bd[:])

            xinT = P.tile([128, 4 * T * BL], F32)
            xinT_v = xinT[:].rearrange("p (hc t b) -> p hc t b", hc=4, t=T, b=BL)
            K_all = P.tile([128, T], F32)
            A_sb = P.tile([128, H], F32)
            hT = P.tile([128, 32], F32)
            hh = P.tile([40, H], F32)
            surp40 = P.tile([40, 1], F32)
            nE20 = P.tile([8, 1], F32)
            errsum = P.tile([128, 1], F32)
            lt8 = P.tile([8, 1], F32)
            sa8 = P.tile([8, 1], F32)
            sleep8 = P.tile([8, 1], F32)
            sp2 = P.tile([8, 1], F32)
            r8 = P.tile([8, 1], F32)
            tmp8 = P.tile([8, 1], F32)
            sa_sb = P.tile([128, 1], F32)
            c_nthr = P.tile([8, 1], F32)

            nc.vector.memset(A_sb[:], 0.0)
            nc.vector.memset(hT[:], 0.0)
            nc.vector.memset(hh[:], 0.0)
            nc.vector.memset(surp40[:], 0.0)
            nc.vector.memset(nE20[:], 0.0)
            nc.vector.memset(c_nthr[:], float(-BASE_THRESH))

            # ---------------- precompute: xin = x @ W_in.T, k = x @ B_proj.T
            with tc.tile_pool(name="pre_sb", bufs=3) as PS, \
                 tc.tile_pool(name="pre_ps", bufs=2, space="PSUM") as PP:
                WinT = [PS.tile([128, H], F32, tag=f"WinT{i}", name=f"WinT{i}") for i in range(2)]
                BpT = [PS.tile([128, R], F32, tag=f"BpT{i}", name=f"BpT{i}") for i in range(2)]
                for dc in range(2):
                    nc.sync.dma_start(WinT[dc][:], d_WinT[dc])
                    nc.sync.dma_start(BpT[dc][:], d_BpT[dc])
                TB = 64  # t-block
                for tb in range(T // TB):
                    rhs = [PS.tile([128, TB * BL], F32, tag=f"xrhs{i}", name=f"xrhs{i}") for i in range(2)]
                    for dc in range(2):
                        nc.sync.dma_start(
                            rhs[dc][:],
                            d_xT[:, dc, tb * TB:(tb + 1) * TB, :])
                    for hc in range(4):
                        ps = PP.tile([128, TB * BL], F32, tag="xps")
                        nc.tensor.matmul(ps[:], WinT[0][:, 128 * hc:128 * (hc + 1)],
                                         rhs[0][:], start=True, stop=False)
                        nc.tensor.matmul(ps[:], WinT[1][:, 128 * hc:128 * (hc + 1)],
                                         rhs[1][:], start=False, stop=True)
                        eng = nc.vector if hc % 2 == 0 else nc.scalar
                        if hc % 2 == 0:
                            nc.vector.tensor_copy(
                                xinT_v[:, hc, tb * TB:(tb + 1) * TB, :], ps[:])
                        else:
                            nc.scalar.copy(
                                xinT_v[:, hc, tb * TB:(tb + 1) * TB, :], ps[:])
                    psk = PP.tile([128, TB * BL], F32, tag="xps")
                    nc.tensor.matmul(psk[0:16, :], BpT[0][:], rhs[0][:],
                                     start=True, stop=False)
                    nc.tensor.matmul(psk[0:16, :], BpT[1][:], rhs[1][:],
                                     start=False, stop=True)
                    kb = PS.tile([16, TB * BL], F32, tag="kb")
                    nc.vector.tensor_copy(kb[:], psk[0:16, :])
                    kb_v = kb[:].rearrange("r (t b) -> r t b", t=TB, b=BL)
                    for b in range(8):
                        nc.sync.dma_start(
                            K_all[16 * b:16 * (b + 1), tb * TB:(tb + 1) * TB],
                            kb_v[:, :, b])

            # ---------------- recurrent loop
            with tc.tile_pool(name="pm", bufs=2, space="PSUM") as PM, \
                 tc.tile_pool(name="pa", bufs=2, space="PSUM") as PA, \
                 tc.tile_pool(name="ph", bufs=2, space="PSUM") as PH, \
                 tc.tile_pool(name="step", bufs=2) as SP, \
                 tc.tile_pool(name="xring", bufs=8) as XR:
                for t in range(nsteps):
                    xt = XR.tile([128, 2, BL], F32, tag="xt")
                    nc.sync.dma_start(xt[:], d_xT[:, :, t, :])

                    pm = PM.tile([128, 512], F32, tag="pm")
                    pa = PA.tile([128, 512], F32, tag="pa")
                    ph = PH.tile([128, 512], F32, tag="ph")

                    K_sel = SP.tile([128, 8], F32, tag="ksel")
                    nc.vector.tensor_scalar(K_sel[:], E_half[:], K_all[:, t:t + 1],
                                            None, AL.mult)

                    # big waves: 4 K-chunks x 4 col groups
                    for kc in range(4):
                        for j in range(4):
                            nc.tensor.matmul(
                                pm[32 * j:32 * j + 8, 0:256],
                                hT[:, 8 * kc:8 * (kc + 1)],
                                WallT[kc][:, 256 * j:256 * (j + 1)],
                                start=(kc == 0), stop=False,
                                tile_position=(0, 32 * j))
                    # xin via identity-rhs transpose-MMs
                    for cx in range(4):
                        j = cx // 2
                        nc.tensor.matmul(
                            pm[32 * j:32 * j + 8, 128 * (cx % 2):128 * (cx % 2 + 1)],
                            xinT_v[:, cx, t, :], I128[:],
                            start=False, stop=False, tile_position=(0, 32 * j))
                    # fast readout into pre (stop for groups 0,1)
                    nc.tensor.matmul(pm[0:8, 0:256], K_sel[:], A_sb[:, 0:256],
                                     start=False, stop=True, tile_position=(0, 0))
                    nc.tensor.matmul(pm[32:40, 0:256], K_sel[:], A_sb[:, 256:512],
                                     start=False, stop=True, tile_position=(0, 32))
                    # -x_t into xhat (group 2)
                    for dc in range(2):
                        nc.tensor.matmul(
                            pm[64:72, 128 * dc:128 * (dc + 1)],
                            xt[:, dc, :], nI128[:],
                            start=False, stop=(dc == 1), tile_position=(0, 64))
                    # b_dec into recon (group 3)
                    nc.tensor.matmul(pm[96:104, 0:256], o18[:], bdec[:],
                                     start=False, stop=True, tile_position=(0, 96))

                    # recon_{t-1} out (psum has no DMA route; bounce via sbuf)
                    if t > 0:
                        rec_sb = SP.tile([8, 256], F32, tag="rec", bufs=3)
                        if t % 2 == 0:
                            nc.scalar.copy(rec_sb[:], pm[96:104, 0:256])
                        else:
                            nc.vector.tensor_copy(rec_sb[:], pm[96:104, 0:256])
                        nc.sync.dma_start(d_y[:, t - 1, :], rec_sb[:])

                    # error chain
                    sqd = SP.tile([128, 256], F32, tag="sqd")
                    nc.scalar.activation(sqd[64:72, :], pm[64:72, 0:256], AF.Square,
                                         accum_out=errsum[64:72, :])
                    nc.tensor.matmul(pm[0:8, 288:289], E_move[64:72, :],
                                     errsum[64:72, :], start=True, stop=True)
                    nc.scalar.activation(surp40[0:8, :], pm[0:8, 288:289], AF.Sigmoid,
                                         bias=nE20[:], scale=float(1.0 / (256 * SURP_TEMP)))
                    nc.scalar.activation(surp40[32:40, :], pm[0:8, 288:289], AF.Sigmoid,
                                         bias=nE20[:], scale=float(1.0 / (256 * SURP_TEMP)))
                    # nE20' = 0.95*nE20 - errsum/256
                    nc.vector.tensor_scalar(tmp8[:], pm[0:8, 288:289],
                                            float(-1.0 / 256.0), None, AL.mult)
                    nc.vector.scalar_tensor_tensor(nE20[:], nE20[:],
                                                   float(1.0 - ERR_SMOOTH),
                                                   tmp8[:], AL.mult, AL.add)
                    # plasticity / sleep chain
                    nc.scalar.activation(r8[:], surp40[0:8, :], AF.Relu,
                                         bias=c_nthr[:])
                    nc.vector.tensor_scalar(lt8[:], surp40[0:8, :], float(MIN_SURP),
                                            None, AL.is_lt)
                    nc.vector.tensor_scalar(sleep8[:], lt8[:], float(-SLEEP_RATE),
                                            1.0, AL.mult, AL.add)
                    nc.vector.tensor_scalar(sa8[:], lt8[:],
                                            float(-SLEEP_RATE * (1.0 - FORGET)),
                                            float(1.0 - FORGET), AL.mult, AL.add)
                    # sp2 = 2*plast*sleep = relu * sleep (2x compensates E_half's kappa)
                    nc.vector.tensor_tensor(sp2[:], r8[:], sleep8[:], AL.mult)
                    # sa broadcast to (b,r) partitions
                    nc.tensor.matmul(pm[0:128, 290:291], E_bT[:], sa8[:],
                                     start=True, stop=True)
                    nc.vector.tensor_copy(sa_sb[:], pm[:, 290:291])
                    # D40 diag
                    D40 = SP.tile([40, 8], F32, tag="d40")
                    nc.vector.scalar_tensor_tensor(D40[:], Es40[:], surp40[:],
                                                   Eo40[:], AL.mult, AL.add)

                    # tanh into hh rows 32:40
                    nc.scalar.activation(hh[32:40, 0:256], pm[0:8, 0:256], AF.Tanh)
                    nc.scalar.activation(hh[32:40, 256:512], pm[32:40, 0:256], AF.Tanh)

                    # h_newT = hh[0:40].T @ D40  -> ph cols 0:32
                    for kc in range(4):
                        nc.tensor.matmul(ph[:, 8 * kc:8 * (kc + 1)],
                                         hh[:, 128 * kc:128 * (kc + 1)], D40[:],
                                         start=True, stop=True)
                    hT_n = hT
                    nc.vector.tensor_copy(hT_n[:], ph[:, 0:32])

                    # back-transpose h_new -> hh rows 0:8
                    for hc in range(4):
                        ro = 64 if hc < 2 else 96
                        co = 128 + 128 * (hc % 2)
                        nc.tensor.matmul(ph[ro:ro + 8, co:co + 128],
                                         hT_n[:, 8 * hc:8 * (hc + 1)], I128[:],
                                         start=True, stop=True,
                                         tile_position=(0, ro))
                    nc.scalar.copy(hh[0:8, 0:128], ph[64:72, 128:256])
                    nc.scalar.copy(hh[0:8, 128:256], ph[64:72, 256:384])
                    nc.vector.tensor_copy(hh[0:8, 256:384], ph[96:104, 128:256])
                    nc.vector.tensor_copy(hh[0:8, 384:512], ph[96:104, 256:384])

                    # K_blk = (K_sel.T) * sp2  (2x undoes kappa)
                    nc.tensor.matmul(pm[0:8, 298:426], K_sel[:], I128[:],
                                     start=True, stop=True)
                    K_blk = SP.tile([8, 128], F32, tag="kblk")
                    nc.vector.tensor_scalar(K_blk[:], pm[0:8, 298:426], sp2[:],
                                            None, AL.mult)

                    # A outer product + decay update
                    nc.tensor.matmul(pa[:, 0:256], K_blk[:], hh[0:8, 0:256],
                                     start=True, stop=True)
                    nc.tensor.matmul(pa[:, 256:512], K_blk[:], hh[0:8, 256:512],
                                     start=True, stop=True)
                    nc.vector.scalar_tensor_tensor(A_sb[:], A_sb[:], sa_sb[:],
                                                   pa[:], AL.mult, AL.add)

                # final recon for t = nsteps-1
                pmf = PM.tile([128, 512], F32, tag="pm")
                for kc in range(4):
                    nc.tensor.matmul(pmf[96:104, 0:256],
                                     hT[:, 8 * kc:8 * (kc + 1)],
                                     WallT[kc][:, 768:1024],
                                     start=(kc == 0), stop=False,
                                     tile_position=(0, 96))
                nc.tensor.matmul(pmf[96:104, 0:256], o18[:], bdec[:],
                                 start=False, stop=True, tile_position=(0, 96))
                rec_f = SP.tile([8, 256], F32, tag="rec", bufs=3)
                nc.vector.tensor_copy(rec_f[:], pmf[96:104, 0:256])
                nc.sync.dma_start(d_y[:, nsteps - 1, :], rec_f[:])

    nc.finalize()
    return nc


def kernel(x, W_in, W_rec, W_pred, B_proj, W_dec, b_dec, _nsteps=T, _trace=False):
    x = np.asarray(x, np.float32)
    consts = _host_constants(np.asarray(W_in, np.float32),
                             np.asarray(W_rec, np.float32),
                             np.asarray(W_pred, np.float32),
                             np.asarray(B_proj, np.float32),
                             np.asarray(W_dec, np.float32),
                             np.asarray(b_dec, np.float32))
    key = _nsteps
    if key not in _CACHE:
        _CACHE[key] = _build(_nsteps)
    nc = _CACHE[key]

    in_maps = []
    for c in range(NCORES):
        xs = x[c * BL:(c + 1) * BL]                      # [8, T, D]
        xT = np.ascontiguousarray(
            xs.transpose(2, 1, 0).reshape(2, 128, T, BL).transpose(1, 0, 2, 3))
        m = dict(consts)
        m["xT"] = xT
        in_maps.append(m)

    res = run_bass_kernel_spmd(nc, in_maps, core_ids=list(range(NCORES)),
                               trace=_trace)
    y = np.concatenate([res.results[c]["y"] for c in range(NCORES)], axis=0)
    if _trace:
        kernel.last_exec_time_ns = res.exec_time_ns
    return y


# revision 8
# speedup vs baseline: 11.4899x; 11.4899x over previous
"""DREAMReconstructor Trainium2 kernel.

Strategy: data-parallel over batch across 8 NeuronCores (8 rows/core).
Per core, a fully-unrolled 256-step recurrence where every matmul keeps
batch (M=8) as the PE stationary-column dim:

  - one combined weight matrix W_all = [W_rec; W_pred; W_dec] streamed as
    the moving operand, 4-way column-tiled across PE quadrants so the four
    K-chunks' waves run concurrently (out rows 32j hold h@W_rec (pre),
    h@W_pred (xhat), h@W_dec (recon)),
  - x@W_in.T precomputed for all t (xin), accumulated into the pre psum
    via identity-rhs transpose matmuls (no extra vector add),
  - -x_t accumulated into the xhat psum the same way, so the error
    reduction is a single ACT Square with accum_out,
  - low-rank fast weights A [(b,r)=128, h=512] live in SBUF; readout is a
    masked-selector matmul accumulated straight into pre; the Hebbian
    outer-product update is one K=8 matmul into psum plus a single
    fused scalar_tensor_tensor decay+add,
  - the leaky h update h' = tanh(pre)*iv + h*(1-iv) is computed by the PE
    via a stacked [40,512] stationary against a per-step diagonal matrix
    built with one fused vector op.
"""
import sys
import numpy as np

sys.path.insert(0, "/opt/trn_rl_repo")

import concourse.bass as bass
import concourse.tile as tile
from concourse import bacc, mybir
from concourse.bass_utils import run_bass_kernel_spmd

F32 = mybir.dt.float32

B, T, D, H, R = 64, 256, 256, 512, 16
NCORES = 8
BL = B // NCORES  # 8 rows per core

FORGET = 0.005
BASE_PLAST = 0.5
BASE_THRESH = 0.3
SURP_TEMP = 0.05
ERR_SMOOTH = 0.05
LTC_TAU = 5.0
LTC_SCALE = 5.0
KAPPA = 0.5
SLEEP_RATE = 0.01
MIN_SURP = 0.15

_CACHE = {}


def _host_constants(W_in, W_rec, W_pred, B_proj, W_dec, b_dec):
    c = {}
    W_all = np.concatenate([W_rec, W_pred, W_dec], axis=0)          # [1024, 512]
    c["W_allT"] = np.ascontiguousarray(W_all.T.reshape(4, 128, 1024, order="F")
                                       if False else
                                       W_all.T.reshape(512, 1024).reshape(4, 128, 1024))
    c["W_inT"] = np.ascontiguousarray(W_in.T.reshape(2, 128, H))     # [256,512]->[2,128,512]
    c["BprojT"] = np.ascontiguousarray(B_proj.T.reshape(2, 128, R))  # [256,16]->[2,128,16]
    c["I128"] = np.eye(128, dtype=np.float32)
    c["nI128"] = -np.eye(128, dtype=np.float32)
    e = np.zeros((128, 8), np.float32)
    for p in range(128):
        e[p, p // 16] = KAPPA                                        # kappa folded into readout
    c["E_half"] = e
    c["E_bT"] = np.ascontiguousarray((e.T != 0).astype(np.float32))  # [8,128] 0/1
    em = np.zeros((128, 8), np.float32)
    for j in range(8):
        em[64 + j, j] = 1.0
    c["E_move"] = em
    es = np.zeros((40, 8), np.float32)
    eo = np.zeros((40, 8), np.float32)
    for b in range(8):
        es[b, b] = -1.0          # h_b side: om = 0.8 - surp
        eo[b, b] = 0.8
        es[32 + b, b] = 1.0      # th side: iv = 0.2 + surp
        eo[32 + b, b] = 0.2
    c["Esign40"] = es
    c["Eoff40"] = eo
    c["ones18"] = np.ones((1, 8), np.float32)
    c["bdec_row"] = np.ascontiguousarray(b_dec.reshape(1, D))
    return c


def _build(nsteps):
    nc = bacc.Bacc("TRN2", target_bir_lowering=False, debug=False, num_devices=NCORES)

    d_xT = nc.dram_tensor("xT", [128, 2, T, BL], F32, kind="ExternalInput")
    d_WallT = nc.dram_tensor("W_allT", [4, 128, 1024], F32, kind="ExternalInput")
    d_WinT = nc.dram_tensor("W_inT", [2, 128, H], F32, kind="ExternalInput")
    d_BpT = nc.dram_tensor("BprojT", [2, 128, R], F32, kind="ExternalInput")
    d_I = nc.dram_tensor("I128", [128, 128], F32, kind="ExternalInput")
    d_nI = nc.dram_tensor("nI128", [128, 128], F32, kind="ExternalInput")
    d_Eh = nc.dram_tensor("E_half", [128, 8], F32, kind="ExternalInput")
    d_EbT = nc.dram_tensor("E_bT", [8, 128], F32, kind="ExternalInput")
    d_Emv = nc.dram_tensor("E_move", [128, 8], F32, kind="ExternalInput")
    d_Es = nc.dram_tensor("Esign40", [40, 8], F32, kind="ExternalInput")
    d_Eo = nc.dram_tensor("Eoff40", [40, 8], F32, kind="ExternalInput")
    d_o18 = nc.dram_tensor("ones18", [1, 8], F32, kind="ExternalInput")
    d_bd = nc.dram_tensor("bdec_row", [1, D], F32, kind="ExternalInput")
    d_y = nc.dram_tensor("y", [BL, T, D], F32, kind="ExternalOutput")

    AL = mybir.AluOpType
    AF = mybir.ActivationFunctionType

    with tile.TileContext(nc) as tc:
        with tc.tile_pool(name="persist", bufs=1) as P:
            # persistent tiles
            WallT = [P.tile([128, 1024], F32, tag=f"WallT{i}", name=f"WallT{i}") for i in range(4)]
            for kc in range(4):
                nc.sync.dma_start(WallT[kc][:], d_WallT[kc])
            I128 = P.tile([128, 128], F32); nc.sync.dma_start(I128[:], d_I[:])
            nI128 = P.tile([128, 128], F32); nc.sync.dma_start(nI128[:], d_nI[:])
            E_half = P.tile([128, 8], F32); nc.sync.dma_start(E_half[:], d_Eh[:])
            E_bT = P.tile([8, 128], F32); nc.sync.dma_start(E_bT[:], d_EbT[:])
            E_move = P.tile([128, 8], F32); nc.sync.dma_start(E_move[:], d_Emv[:])
            Es40 = P.tile([40, 8], F32); nc.sync.dma_start(Es40[:], d_Es[:])
            Eo40 = P.tile([40, 8], F32); nc.sync.dma_start(Eo40[:], d_Eo[:])
            o18 = P.tile([1, 8], F32); nc.sync.dma_start(o18[:], d_o18[:])
            bdec = P.tile([1, D], F32); nc.sync.dma_start(bdec[:], d_bd[:])

            xinT = P.tile([128, 4 * T * BL], F32)
            xinT_v = xinT[:].rearrange("p (hc t b) -> p hc t b", hc=4, t=T, b=BL)
            K_all = P.tile([128, T], F32)
            A_sb = P.tile([128, H], F32)
            hT = P.tile([128, 32], F32)
            hh = P.tile([40, H], F32)
            surp40 = P.tile([40, 1], F32)
            nE20 = P.tile([8, 1], F32)
            errsum = P.tile([128, 1], F32)
            lt8 = P.tile([8, 1], F32)
            sa8 = P.tile([8, 1], F32)
            sleep8 = P.tile([8, 1], F32)
            sp2 = P.tile([8, 1], F32)
            r8 = P.tile([8, 1], F32)
            tmp8 = P.tile([8, 1], F32)
            sa_sb = P.tile([128, 1], F32)
            c_nthr = P.tile([8, 1], F32)

            nc.vector.memset(A_sb[:], 0.0)
            nc.vector.memset(hT[:], 0.0)
            nc.vector.memset(hh[:], 0.0)
            nc.vector.memset(surp40[:], 0.0)
            nc.vector.memset(nE20[:], 0.0)
            nc.vector.memset(c_nthr[:], float(-BASE_THRESH))

            # ---------------- precompute: xin = x @ W_in.T, k = x @ B_proj.T
            with tc.tile_pool(name="pre_sb", bufs=3) as PS, \
                 tc.tile_pool(name="pre_ps", bufs=2, space="PSUM") as PP:
                WinT = [PS.tile([128, H], F32, tag=f"WinT{i}", name=f"WinT{i}") for i in range(2)]
                BpT = [PS.tile([128, R], F32, tag=f"BpT{i}", name=f"BpT{i}") for i in range(2)]
                for dc in range(2):
                    nc.sync.dma_start(WinT[dc][:], d_WinT[dc])
                    nc.sync.dma_start(BpT[dc][:], d_BpT[dc])
                TB = 64  # t-block
                for tb in range(T // TB):
                    rhs = [PS.tile([128, TB * BL], F32, tag=f"xrhs{i}", name=f"xrhs{i}") for i in range(2)]
                    for dc in range(2):
                        nc.sync.dma_start(
                            rhs[dc][:],
                            d_xT[:, dc, tb * TB:(tb + 1) * TB, :])
                    for hc in range(4):
                        ps = PP.tile([128, TB * BL], F32, tag="xps")
                        nc.tensor.matmul(ps[:], WinT[0][:, 128 * hc:128 * (hc + 1)],
                                         rhs[0][:], start=True, stop=False)
                        nc.tensor.matmul(ps[:], WinT[1][:, 128 * hc:128 * (hc + 1)],
                                         rhs[1][:], start=False, stop=True)
                        eng = nc.vector if hc % 2 == 0 else nc.scalar
                        if hc % 2 == 0:
                            nc.vector.tensor_copy(
                                xinT_v[:, hc, tb * TB:(tb + 1) * TB, :], ps[:])
                        else:
                            nc.scalar.copy(
                                xinT_v[:, hc, tb * TB:(tb + 1) * TB, :], ps[:])
                    psk = PP.tile([128, TB * BL], F32, tag="xps")
                    nc.tensor.matmul(psk[0:16, :], BpT[0][:], rhs[0][:],
                                     start=True, stop=False)
                    nc.tensor.matmul(psk[0:16, :], BpT[1][:], rhs[1][:],
                                     start=False, stop=True)
                    kb = PS.tile([16, TB * BL], F32, tag="kb")
                    nc.vector.tensor_copy(kb[:], psk[0:16, :])
                    kb_v = kb[:].rearrange("r (t b) -> r t b", t=TB, b=BL)
                    for b in range(8):
                        nc.sync.dma_start(
                            K_all[16 * b:16 * (b + 1), tb * TB:(tb + 1) * TB],
                            kb_v[:, :, b])

            # ---------------- recurrent loop
            with tc.tile_pool(name="pm", bufs=2, space="PSUM") as PM, \
                 tc.tile_pool(name="pa", bufs=2, space="PSUM") as PA, \
                 tc.tile_pool(name="ph", bufs=2, space="PSUM") as PH, \
                 tc.tile_pool(name="step", bufs=2) as SP, \
                 tc.tile_pool(name="xring", bufs=8) as XR:
                for t in range(nsteps):
                    xt = XR.tile([128, 2, BL], F32, tag="xt")
                    nc.sync.dma_start(xt[:], d_xT[:, :, t, :])

                    pm = PM.tile([128, 512], F32, tag="pm")
                    pa = PA.tile([128, 512], F32, tag="pa")
                    ph = PH.tile([128, 512], F32, tag="ph")

                    K_sel = SP.tile([128, 8], F32, tag="ksel")
                    nc.vector.tensor_scalar(K_sel[:], E_half[:], K_all[:, t:t + 1],
                                            None, AL.mult)

                    # big waves: 4 K-chunks x 4 col groups
                    for kc in range(4):
                        for j in range(4):
                            nc.tensor.matmul(
                                pm[32 * j:32 * j + 8, 0:256],
                                hT[:, 8 * kc:8 * (kc + 1)],
                                WallT[kc][:, 256 * j:256 * (j + 1)],
                                start=(kc == 0), stop=False,
                                tile_position=(0, 32 * j))
                    # xin via identity-rhs transpose-MMs
                    for cx in range(4):
                        j = cx // 2
                        nc.tensor.matmul(
                            pm[32 * j:32 * j + 8, 128 * (cx % 2):128 * (cx % 2 + 1)],
                            xinT_v[:, cx, t, :], I128[:],
                            start=False, stop=False, tile_position=(0, 32 * j))
                    # fast readout into pre (stop for groups 0,1)
                    nc.tensor.matmul(pm[0:8, 0:256], K_sel[:], A_sb[:, 0:256],
                                     start=False, stop=True, tile_position=(0, 0))
                    nc.tensor.matmul(pm[32:40, 0:256], K_sel[:], A_sb[:, 256:512],
                                     start=False, stop=True, tile_position=(0, 32))
                    # -x_t into xhat (group 2)
                    for dc in range(2):
                        nc.tensor.matmul(
                            pm[64:72, 128 * dc:128 * (dc + 1)],
                            xt[:, dc, :], nI128[:],
                            start=False, stop=(dc == 1), tile_position=(0, 64))
                    # b_dec into recon (group 3)
                    nc.tensor.matmul(pm[96:104, 0:256], o18[:], bdec[:],
                                     start=False, stop=True, tile_position=(0, 96))

                    # recon_{t-1} out (psum has no DMA route; bounce via sbuf)
                    if t > 0:
                        rec_sb = SP.tile([8, 256], F32, tag="rec", bufs=3)
                        if t % 2 == 0:
                            nc.scalar.copy(rec_sb[:], pm[96:104, 0:256])
                        else:
                            nc.vector.tensor_copy(rec_sb[:], pm[96:104, 0:256])
                        nc.sync.dma_start(d_y[:, t - 1, :], rec_sb[:])

                    # error chain
                    sqd = SP.tile([128, 256], F32, tag="sqd")
                    nc.scalar.activation(sqd[64:72, :], pm[64:72, 0:256], AF.Square,
                                         accum_out=errsum[64:72, :])
                    nc.tensor.matmul(pm[0:8, 288:289], E_move[64:72, :],
                                     errsum[64:72, :], start=True, stop=True)
                    nc.scalar.activation(surp40[0:8, :], pm[0:8, 288:289], AF.Sigmoid,
                                         bias=nE20[:], scale=float(1.0 / (256 * SURP_TEMP)))
                    nc.scalar.activation(surp40[32:40, :], pm[0:8, 288:289], AF.Sigmoid,
                                         bias=nE20[:], scale=float(1.0 / (256 * SURP_TEMP)))
                    # nE20' = 0.95*nE20 - errsum/256
                    nc.vector.tensor_scalar(tmp8[:], pm[0:8, 288:289],
                                            float(-1.0 / 256.0), None, AL.mult)
                    nc.vector.scalar_tensor_tensor(nE20[:], nE20[:],
                                                   float(1.0 - ERR_SMOOTH),
                                                   tmp8[:], AL.mult, AL.add)
                    # plasticity / sleep chain
                    nc.scalar.activation(r8[:], surp40[0:8, :], AF.Relu,
                                         bias=c_nthr[:])
                    nc.vector.tensor_scalar(lt8[:], surp40[0:8, :], float(MIN_SURP),
                                            None, AL.is_lt)
                    nc.vector.tensor_scalar(sleep8[:], lt8[:], float(-SLEEP_RATE),
                                            1.0, AL.mult, AL.add)
                    nc.vector.tensor_scalar(sa8[:], lt8[:],
                                            float(-SLEEP_RATE * (1.0 - FORGET)),
                                            float(1.0 - FORGET), AL.mult, AL.add)
                    # sp2 = 2*plast*sleep = relu * sleep (2x compensates E_half's kappa)
                    nc.vector.tensor_tensor(sp2[:], r8[:], sleep8[:], AL.mult)
                    # sa broadcast to (b,r) partitions
                    nc.tensor.matmul(pm[0:128, 290:291], E_bT[:], sa8[:],
                                     start=True, stop=True)
                    nc.vector.tensor_copy(sa_sb[:], pm[:, 290:291])
                    # D40 diag
                    D40 = SP.tile([40, 8], F32, tag="d40")
                    nc.vector.scalar_tensor_tensor(D40[:], Es40[:], surp40[:],
                                                   Eo40[:], AL.mult, AL.add)

                    # tanh into hh rows 32:40
                    nc.scalar.activation(hh[32:40, 0:256], pm[0:8, 0:256], AF.Tanh)
                    nc.scalar.activation(hh[32:40, 256:512], pm[32:40, 0:256], AF.Tanh)

                    # h_newT = hh[0:40].T @ D40  -> ph cols 0:32
                    for kc in range(4):
                        nc.tensor.matmul(ph[:, 8 * kc:8 * (kc + 1)],
                                         hh[:, 128 * kc:128 * (kc + 1)], D40[:],
                                         start=True, stop=True)
                    hT_n = hT
                    nc.vector.tensor_copy(hT_n[:], ph[:, 0:32])

                    # back-transpose h_new -> hh rows 0:8
                    for hc in range(4):
                        ro = 64 if hc < 2 else 96
                        co = 128 + 128 * (hc % 2)
                        nc.tensor.matmul(ph[ro:ro + 8, co:co + 128],
                                         hT_n[:, 8 * hc:8 * (hc + 1)], I128[:],
                                         start=True, stop=True,
                                         tile_position=(0, ro))
                    nc.scalar.copy(hh[0:8, 0:128], ph[64:72, 128:256])
                    nc.scalar.copy(hh[0:8, 128:256], ph[64:72, 256:384])
                    nc.vector.tensor_copy(hh[0:8, 256:384], ph[96:104, 128:256])
                    nc.vector.tensor_copy(hh[0:8, 384:512], ph[96:104, 256:384])

                    # K_blk = (K_sel.T) * sp2  (2x undoes kappa)
                    nc.tensor.matmul(pm[0:8, 298:426], K_sel[:], I128[:],
                                     start=True, stop=True)
                    K_blk = SP.tile([8, 128], F32, tag="kblk")
                    nc.vector.tensor_scalar(K_blk[:], pm[0:8, 298:426], sp2[:],
                                            None, AL.mult)

                    # A outer product + decay update
                    nc.tensor.matmul(pa[:, 0:256], K_blk[:], hh[0:8, 0:256],
                                     start=True, stop=True)
                    nc.tensor.matmul(pa[:, 256:512], K_blk[:], hh[0:8, 256:512],
                                     start=True, stop=True)
                    nc.vector.scalar_tensor_tensor(A_sb[:], A_sb[:], sa_sb[:],
                                                   pa[:], AL.mult, AL.add)

                # final recon for t = nsteps-1
                pmf = PM.tile([128, 512], F32, tag="pm")
                for kc in range(4):
                    nc.tensor.matmul(pmf[96:104, 0:256],
                                     hT[:, 8 * kc:8 * (kc + 1)],
                                     WallT[kc][:, 768:1024],
                                     start=(kc == 0), stop=False,
                                     tile_position=(0, 96))
                nc.tensor.matmul(pmf[96:104, 0:256], o18[:], bdec[:],
                                 start=False, stop=True, tile_position=(0, 96))
                rec_f = SP.tile([8, 256], F32, tag="rec", bufs=3)
                nc.vector.tensor_copy(rec_f[:], pmf[96:104, 0:256])
                nc.sync.dma_start(d_y[:, nsteps - 1, :], rec_f[:])

    nc.finalize()
    return nc


def _make_runner(nc):
    """Persistent jitted SPMD executor (mirrors bass2jax.run_bass_via_pjrt,
    but reusable across calls so the NEFF stays loaded on the devices)."""
    import jax
    from jax.experimental.shard_map import shard_map
    from jax.sharding import Mesh, PartitionSpec
    from concourse import bass2jax
    from concourse import mybir as mb

    bass2jax.install_neuronx_cc_hook()

    partition_name = (nc.partition_id_tensor.name
                      if nc.partition_id_tensor else None)
    in_names, out_names, out_avals, zero_outs = [], [], [], []
    for alloc in nc.m.functions[0].allocations:
        if not isinstance(alloc, mb.MemoryLocationSet):
            continue
        name = alloc.memorylocations[0].name
        if alloc.kind == "ExternalInput":
            if name != partition_name:
                in_names.append(name)
        elif alloc.kind == "ExternalOutput":
            out_names.append(name)
            shape = tuple(alloc.tensor_shape)
            dtype = mb.dt.np(alloc.dtype)
            out_avals.append(jax.core.ShapedArray(shape, dtype))
            zero_outs.append(np.zeros(shape, dtype))
    n_params = len(in_names)
    n_outs = len(out_avals)
    all_in_names = list(in_names) + list(out_names)
    if partition_name is not None:
        all_in_names.append(partition_name)

    def _body(*args):
        operands = list(args)
        if partition_name is not None:
            operands.append(bass2jax.partition_id_tensor())
        outs = bass2jax._bass_exec_p.bind(
            *operands,
            out_avals=tuple(out_avals),
            in_names=tuple(all_in_names),
            out_names=tuple(out_names),
            lowering_input_output_aliases=(),
            sim_require_finite=True,
            sim_require_nnan=True,
            nc=nc,
        )
        return tuple(outs)

    devices = jax.devices()[:NCORES]
    mesh = Mesh(np.asarray(devices), ("core",))
    in_specs = (PartitionSpec("core"),) * (n_params + n_outs)
    out_specs = (PartitionSpec("core"),) * len(out_names)
    donate = tuple(range(n_params, n_params + n_outs))
    sharded = jax.jit(
        shard_map(_body, mesh=mesh, in_specs=in_specs, out_specs=out_specs,
                  check_rep=False),
        donate_argnums=donate, keep_unused=True)

    def run(in_maps):
        concat_in = [
            np.concatenate([np.asarray(in_maps[c][n]) for c in range(NCORES)],
                           axis=0)
            for n in in_names
        ]
        concat_zeros = [
            np.zeros((NCORES * z.shape[0], *z.shape[1:]), z.dtype)
            for z in zero_outs
        ]
        out_arrs = sharded(*concat_in, *concat_zeros)
        return [
            {n: np.asarray(out_arrs[i]).reshape(NCORES, *out_avals[i].shape)[c]
             for i, n in enumerate(out_names)}
            for c in range(NCORES)
        ]

    return run


def kernel(x, W_in, W_rec, W_pred, B_proj, W_dec, b_dec, _nsteps=T, _trace=False):
    x = np.asarray(x, np.float32)
    consts = _host_constants(np.asarray(W_in, np.float32),
                             np.asarray(W_rec, np.float32),
                             np.asarray(W_pred, np.float32),
                             np.asarray(B_proj, np.float32),
                             np.asarray(W_dec, np.float32),
                             np.asarray(b_dec, np.float32))
    key = _nsteps
    if key not in _CACHE:
        nc = _build(_nsteps)
        _CACHE[key] = _make_runner(nc)
    run = _CACHE[key]

    in_maps = []
    for c in range(NCORES):
        xs = x[c * BL:(c + 1) * BL]                      # [8, T, D]
        xT = np.ascontiguousarray(
            xs.transpose(2, 1, 0).reshape(2, 128, T, BL).transpose(1, 0, 2, 3))
        m = dict(consts)
        m["xT"] = xT
        in_maps.append(m)

    results = run(in_maps)
    y = np.concatenate([results[c]["y"] for c in range(NCORES)], axis=0)
    return y


# revision 21
# speedup vs baseline: 563.9172x; 49.0792x over previous
"""DREAMReconstructor Trainium2 kernel.

Strategy: data-parallel over batch across 8 NeuronCores (8 rows/core).
Per core, a fully-unrolled 256-step recurrence where every matmul keeps
batch (M=8) as the PE stationary-column dim:

  - one combined weight matrix W_all = [W_rec; W_pred; W_dec] streamed as
    the moving operand, 4-way column-tiled across PE quadrants so the four
    K-chunks' waves run concurrently (out rows 32j hold h@W_rec (pre),
    h@W_pred (xhat), h@W_dec (recon)),
  - x@W_in.T precomputed for all t (xin), accumulated into the pre psum
    via identity-rhs transpose matmuls (no extra vector add),
  - -x_t accumulated into the xhat psum the same way, so the error
    reduction is a single ACT Square with accum_out,
  - low-rank fast weights A [(b,r)=128, h=512] live in SBUF; readout is a
    masked-selector matmul accumulated straight into pre; the Hebbian
    outer-product update is one K=8 matmul into psum plus a single
    fused scalar_tensor_tensor decay+add,
  - the leaky h update h' = tanh(pre)*iv + h*(1-iv) is computed by the PE
    via a stacked [40,512] stationary against a per-step diagonal matrix
    built with one fused vector op.
"""
import sys
import numpy as np

sys.path.insert(0, "/opt/trn_rl_repo")

import concourse.bass as bass
import concourse.tile as tile
from concourse import bacc, mybir
from concourse.bass_utils import run_bass_kernel_spmd

F32 = mybir.dt.float32

B, T, D, H, R = 64, 256, 256, 512, 16
NCORES = 8
BL = B // NCORES  # 8 rows per core

FORGET = 0.005
BASE_PLAST = 0.5
BASE_THRESH = 0.3
SURP_TEMP = 0.05
ERR_SMOOTH = 0.05
LTC_TAU = 5.0
LTC_SCALE = 5.0
KAPPA = 0.5
SLEEP_RATE = 0.01
MIN_SURP = 0.15

_CACHE = {}


def _host_constants(W_in, W_rec, W_pred, B_proj, W_dec, b_dec):
    c = {}
    W_all = np.concatenate([W_rec, W_pred, W_dec], axis=0)          # [1024, 512]
    c["W_allT"] = np.ascontiguousarray(W_all.T.reshape(4, 128, 1024, order="F")
                                       if False else
                                       W_all.T.reshape(512, 1024).reshape(4, 128, 1024))
    c["W_inT"] = np.ascontiguousarray(W_in.T.reshape(2, 128, H))     # [256,512]->[2,128,512]
    c["BprojT"] = np.ascontiguousarray(B_proj.T.reshape(2, 128, R))  # [256,16]->[2,128,16]
    c["I128"] = np.eye(128, dtype=np.float32)
    em = np.zeros((128, 8), np.float32)
    for j in range(8):
        em[64 + j, j] = 1.0
    c["E_move"] = em
    e = np.zeros((128, 8), np.float32)
    for p in range(128):
        e[p, p // 16] = KAPPA                                        # kappa folded into readout
    c["E_half"] = e
    c["E_bT"] = np.ascontiguousarray((e.T != 0).astype(np.float32))  # [8,128] 0/1
    es = np.zeros((40, 8), np.float32)
    eo = np.zeros((40, 8), np.float32)
    for b in range(8):
        es[b, b] = -1.0          # h_b side: om = 0.8 - surp
        eo[b, b] = 0.8
        es[32 + b, b] = 1.0      # th side: iv = 0.2 + surp
        eo[32 + b, b] = 0.2
    c["Esign40"] = es
    c["Eoff40"] = eo
    return c


def _build(nsteps):
    nc = bacc.Bacc("TRN2", target_bir_lowering=False, debug=False, num_devices=NCORES)

    d_xT = nc.dram_tensor("xT", [128, 2, T, BL], F32, kind="ExternalInput")
    d_WallT = nc.dram_tensor("W_allT", [4, 128, 1024], F32, kind="ExternalInput")
    d_WinT = nc.dram_tensor("W_inT", [2, 128, H], F32, kind="ExternalInput")
    d_BpT = nc.dram_tensor("BprojT", [2, 128, R], F32, kind="ExternalInput")
    d_I = nc.dram_tensor("I128", [128, 128], F32, kind="ExternalInput")
    d_nxT = nc.dram_tensor("nxT", [128, 2, T, BL], F32, kind="ExternalInput")
    d_Eh = nc.dram_tensor("E_half", [128, 8], F32, kind="ExternalInput")
    d_Emv = nc.dram_tensor("E_move", [128, 8], F32, kind="ExternalInput")
    d_EbT = nc.dram_tensor("E_bT", [8, 128], F32, kind="ExternalInput")
    d_Es = nc.dram_tensor("Esign40", [40, 8], F32, kind="ExternalInput")
    d_Eo = nc.dram_tensor("Eoff40", [40, 8], F32, kind="ExternalInput")
    d_y = nc.dram_tensor("y", [BL, T, D], F32, kind="ExternalOutput")

    AL = mybir.AluOpType
    AF = mybir.ActivationFunctionType

    with tile.TileContext(nc) as tc:
        with tc.tile_pool(name="persist", bufs=1) as P:
            # persistent tiles
            WallT = [P.tile([128, 1024], F32, tag=f"WallT{i}", name=f"WallT{i}") for i in range(4)]
            for kc in range(4):
                nc.sync.dma_start(WallT[kc][:], d_WallT[kc])
            I128 = P.tile([128, 128], F32); nc.sync.dma_start(I128[:], d_I[:])
            E_half = P.tile([128, 8], F32); nc.sync.dma_start(E_half[:], d_Eh[:])
            E_move = P.tile([128, 8], F32); nc.sync.dma_start(E_move[:], d_Emv[:])
            E_bT = P.tile([8, 128], F32); nc.sync.dma_start(E_bT[:], d_EbT[:])
            Es40 = P.tile([40, 8], F32); nc.sync.dma_start(Es40[:], d_Es[:])
            Eo40 = P.tile([40, 8], F32); nc.sync.dma_start(Eo40[:], d_Eo[:])

            xinT = P.tile([128, 4 * T * BL], F32)
            xinT_v = xinT[:].rearrange("p (hc t b) -> p hc t b", hc=4, t=T, b=BL)
            K_all = P.tile([128, T], F32)
            A_sb = P.tile([128, H], F32)
            hT = P.tile([128, 32], F32)
            hh = P.tile([40, H], F32)
            surp40 = P.tile([40, 1], F32)
            nE20 = P.tile([8, 1], F32)
            errsum = P.tile([128, 1], F32)
            lt8 = P.tile([8, 1], F32)
            sa8 = P.tile([8, 1], F32)
            sleep8 = P.tile([8, 1], F32)
            sp2 = P.tile([8, 1], F32)
            r8 = P.tile([8, 1], F32)
            tmp8 = P.tile([8, 1], F32)
            sa_sb = P.tile([128, 1], F32)
            c_nthr = P.tile([8, 1], F32)

            nc.vector.memset(A_sb[:], 0.0)
            nc.vector.memset(hT[:], 0.0)
            nc.vector.memset(hh[:], 0.0)
            nc.vector.memset(surp40[:], 0.0)
            nc.vector.memset(nE20[:], 0.0)
            nc.vector.memset(c_nthr[:], float(-BASE_THRESH))

            # ---------------- precompute: xin = x @ W_in.T, k = x @ B_proj.T
            with tc.tile_pool(name="pre_sb", bufs=3) as PS, \
                 tc.tile_pool(name="pre_ps", bufs=2, space="PSUM") as PP:
                WinT = [PS.tile([128, H], F32, tag=f"WinT{i}", name=f"WinT{i}") for i in range(2)]
                BpT = [PS.tile([128, R], F32, tag=f"BpT{i}", name=f"BpT{i}") for i in range(2)]
                for dc in range(2):
                    nc.sync.dma_start(WinT[dc][:], d_WinT[dc])
                    nc.sync.dma_start(BpT[dc][:], d_BpT[dc])
                TB = 64  # t-block
                for tb in range(T // TB):
                    rhs = [PS.tile([128, TB * BL], F32, tag=f"xrhs{i}", name=f"xrhs{i}") for i in range(2)]
                    for dc in range(2):
                        nc.sync.dma_start(
                            rhs[dc][:],
                            d_xT[:, dc, tb * TB:(tb + 1) * TB, :])
                    for hc in range(4):
                        ps = PP.tile([128, TB * BL], F32, tag="xps")
                        nc.tensor.matmul(ps[:], (WinT[0][:, 128 * hc:128 * (hc + 1)]),
                                         (rhs[0][:]), start=True, stop=False)
                        nc.tensor.matmul(ps[:], (WinT[1][:, 128 * hc:128 * (hc + 1)]),
                                         (rhs[1][:]), start=False, stop=True)
                        eng = nc.vector if hc % 2 == 0 else nc.scalar
                        if hc % 2 == 0:
                            nc.vector.tensor_copy(
                                xinT_v[:, hc, tb * TB:(tb + 1) * TB, :], ps[:])
                        else:
                            nc.scalar.copy(
                                xinT_v[:, hc, tb * TB:(tb + 1) * TB, :], ps[:])
                    psk = PP.tile([128, TB * BL], F32, tag="xps")
                    nc.tensor.matmul(psk[0:16, :], (BpT[0][:]), (rhs[0][:]),
                                     start=True, stop=False)
                    nc.tensor.matmul(psk[0:16, :], (BpT[1][:]), (rhs[1][:]),
                                     start=False, stop=True)
                    kb = PS.tile([16, TB * BL], F32, tag="kb")
                    nc.vector.tensor_copy(kb[:], psk[0:16, :])
                    kb_v = kb[:].rearrange("r (t b) -> r t b", t=TB, b=BL)
                    for b in range(8):
                        nc.sync.dma_start(
                            K_all[16 * b:16 * (b + 1), tb * TB:(tb + 1) * TB],
                            kb_v[:, :, b])

            # ---------------- recurrent loop
            with tc.tile_pool(name="pm", bufs=2, space="PSUM") as PM, \
                 tc.tile_pool(name="pa", bufs=2, space="PSUM") as PA, \
                 tc.tile_pool(name="ph", bufs=2, space="PSUM") as PH, \
                 tc.tile_pool(name="pb", bufs=2, space="PSUM") as PB, \
                 tc.tile_pool(name="step", bufs=2) as SP, \
                 tc.tile_pool(name="xring", bufs=8) as XR:
                for t in range(nsteps):
                    xt = XR.tile([128, 2, BL], F32, tag="xt")
                    nc.sync.dma_start(xt[:], d_nxT[:, :, t, :])

                    pm = PM.tile([128, 512], F32, tag="pm")
                    pa = PA.tile([128, 512], F32, tag="pa")
                    ph = PH.tile([128, 64], F32, tag="ph")
                    pb = PB.tile([8, 512], F32, tag="pb")

                    K_sel = SP.tile([128, 8], F32, tag="ksel")
                    nc.vector.tensor_scalar(K_sel[:], E_half[:], K_all[:, t:t + 1],
                                            None, AL.mult)

                    # big waves: 4 K-chunks x 4 concurrent col groups (N=256).
                    # group j=2 (xhat) first within each wave so the error
                    # chain can start as early as possible.
                    for kc in range(4):
                        for j in (2, 0, 1, 3):
                            nc.tensor.matmul(
                                pm[32 * j:32 * j + 8, 0:256],
                                hT[:, 8 * kc:8 * (kc + 1)],
                                WallT[kc][:, 256 * j:256 * (j + 1)],
                                start=(kc == 0), stop=(kc == 3 and j == 3),
                                tile_position=(0, 32 * j))
                    # -x_t into xhat (group 2; transpose-mode needs psum
                    # partition 0, so regular fp32 MMs here)
                    for dc in range(2):
                        nc.tensor.matmul(
                            pm[64:72, 128 * dc:128 * (dc + 1)],
                            xt[:, dc, :], I128[:],
                            start=False, stop=(dc == 1), tile_position=(0, 64))
                    # xin into pre: group 0 can use transpose-mode (psum
                    # partition 0), group 1 must use regular MMs
                    for cx in range(4):
                        j = cx // 2
                        nc.tensor.matmul(
                            pm[32 * j:32 * j + 8, 128 * (cx % 2):128 * (cx % 2 + 1)],
                            xinT_v[:, cx, t, :], I128[:], is_transpose=(j == 0),
                            start=False, stop=False, tile_position=(0, 32 * j))
                    # fast readout closes pre (concurrent groups 0/1)
                    nc.tensor.matmul(pm[0:8, 0:256], K_sel[:], A_sb[:, 0:256],
                                     start=False, stop=True, tile_position=(0, 0))
                    nc.tensor.matmul(pm[32:40, 0:256], K_sel[:], A_sb[:, 256:512],
                                     start=False, stop=True, tile_position=(0, 32))
                    # recon region (group 3) closes with the last wave MM; mark
                    # stop via a zero-cost dummy? not needed: waves j=3 kc=3 is
                    # the last writer; set stop there instead.

                    # recon_{t-1} out (psum has no DMA route; bounce via sbuf)
                    if t > 0:
                        rec_sb = SP.tile([8, 256], F32, tag="rec", bufs=3)
                        if t % 2 == 0:
                            nc.scalar.copy(rec_sb[:], pm[96:104, 0:256])
                        else:
                            nc.vector.tensor_copy(rec_sb[:], pm[96:104, 0:256])
                        nc.sync.dma_start(d_y[:, t - 1, :], rec_sb[:])

                    # error chain: errsum = sum((xhat-x_t)^2) at rows 64:72
                    sqd = SP.tile([128, 256], F32, tag="sqd")
                    nc.scalar.activation(sqd[64:72, :], pm[64:72, 0:256], AF.Square,
                                         accum_out=errsum[64:72, :])
                    nc.tensor.matmul(pm[0:8, 384:385], E_move[64:72, :],
                                     errsum[64:72, :], start=True, stop=True)
                    nc.scalar.activation(surp40[0:8, :], pm[0:8, 384:385], AF.Sigmoid,
                                         bias=nE20[:], scale=float(1.0 / (256 * SURP_TEMP)))
                    nc.scalar.activation(surp40[32:40, :], pm[0:8, 384:385], AF.Sigmoid,
                                         bias=nE20[:], scale=float(1.0 / (256 * SURP_TEMP)))
                    # nE20' = 0.95*nE20 - errsum/256
                    nc.vector.tensor_scalar(tmp8[:], pm[0:8, 384:385],
                                            float(-1.0 / 256.0), None, AL.mult)
                    nc.vector.scalar_tensor_tensor(nE20[:], nE20[:],
                                                   float(1.0 - ERR_SMOOTH),
                                                   tmp8[:], AL.mult, AL.add)
                    # plasticity / sleep chain
                    nc.scalar.activation(r8[:], surp40[0:8, :], AF.Relu,
                                         bias=c_nthr[:])
                    nc.vector.tensor_scalar(lt8[:], surp40[0:8, :], float(MIN_SURP),
                                            None, AL.is_lt)
                    nc.vector.tensor_scalar(sleep8[:], lt8[:], float(-SLEEP_RATE),
                                            1.0, AL.mult, AL.add)
                    nc.vector.tensor_scalar(sa8[:], lt8[:],
                                            float(-SLEEP_RATE * (1.0 - FORGET)),
                                            float(1.0 - FORGET), AL.mult, AL.add)
                    # sp2 = 2*plast*sleep = relu * sleep (2x undoes E_half's kappa)
                    nc.vector.tensor_tensor(sp2[:], r8[:], sleep8[:], AL.mult)
                    # sa broadcast to (b,r) partitions -> ph col 32
                    nc.tensor.matmul(ph[0:128, 32:33], E_bT[:], sa8[:],
                                     start=True, stop=True)
                    nc.vector.tensor_copy(sa_sb[:], ph[:, 32:33])
                    # D40 diag
                    D40 = SP.tile([40, 8], F32, tag="d40")
                    nc.vector.scalar_tensor_tensor(D40[:], Es40[:], surp40[:],
                                                   Eo40[:], AL.mult, AL.add)

                    # tanh into hh rows 32:40
                    nc.scalar.activation(hh[32:40, 0:256], pm[0:8, 0:256], AF.Tanh)
                    nc.scalar.activation(hh[32:40, 256:512], pm[32:40, 0:256], AF.Tanh)

                    # h_newT = hh[0:40].T @ D40  -> ph cols 0:32
                    for kc in range(4):
                        nc.tensor.matmul(ph[:, 8 * kc:8 * (kc + 1)],
                                         hh[:, 128 * kc:128 * (kc + 1)], D40[:],
                                         start=True, stop=True)
                    hT_n = hT
                    nc.vector.tensor_copy(hT_n[:], ph[:, 0:32])

                    # back-transpose h_new -> pb rows 0:8 (transpose-mode,
                    # untiled, own bank)
                    for hc in range(4):
                        nc.tensor.matmul(pb[0:8, 128 * hc:128 * (hc + 1)],
                                         hT_n[:, 8 * hc:8 * (hc + 1)], I128[:],
                                         is_transpose=True, start=True, stop=True)
                    nc.scalar.copy(hh[0:8, 0:256], pb[0:8, 0:256])
                    nc.vector.tensor_copy(hh[0:8, 256:512], pb[0:8, 256:512])

                    # K_blk = (K_sel.T) * sp2 (transpose-mode into pm spare)
                    nc.tensor.matmul(pm[0:8, 256:384], K_sel[:], I128[:],
                                     is_transpose=True, start=True, stop=True)
                    K_blk = SP.tile([8, 128], F32, tag="kblk")
                    nc.vector.tensor_scalar(K_blk[:], pm[0:8, 256:384], sp2[:],
                                            None, AL.mult)

                    # A outer product + decay update
                    nc.tensor.matmul(pa[:, 0:512], K_blk[:], hh[0:8, 0:512],
                                     start=True, stop=True)
                    nc.vector.scalar_tensor_tensor(A_sb[:], A_sb[:], sa_sb[:],
                                                   pa[:], AL.mult, AL.add)

                # final recon for t = nsteps-1
                pmf = PM.tile([128, 512], F32, tag="pm")
                for kc in range(4):
                    nc.tensor.matmul(pmf[96:104, 0:256],
                                     hT[:, 8 * kc:8 * (kc + 1)],
                                     WallT[kc][:, 768:1024],
                                     start=(kc == 0), stop=(kc == 3),
                                     tile_position=(0, 96))
                rec_f = SP.tile([8, 256], F32, tag="rec", bufs=3)
                nc.vector.tensor_copy(rec_f[:], pmf[96:104, 0:256])
                nc.sync.dma_start(d_y[:, nsteps - 1, :], rec_f[:])

    nc.finalize()
    return nc


def _make_runner(nc):
    """Persistent jitted SPMD executor (mirrors bass2jax.run_bass_via_pjrt,
    but reusable across calls so the NEFF stays loaded on the devices)."""
    import jax
    from jax.experimental.shard_map import shard_map
    from jax.sharding import Mesh, PartitionSpec
    from concourse import bass2jax
    from concourse import mybir as mb

    bass2jax.install_neuronx_cc_hook()

    partition_name = (nc.partition_id_tensor.name
                      if nc.partition_id_tensor else None)
    in_names, out_names, out_avals, zero_outs = [], [], [], []
    for alloc in nc.m.functions[0].allocations:
        if not isinstance(alloc, mb.MemoryLocationSet):
            continue
        name = alloc.memorylocations[0].name
        if alloc.kind == "ExternalInput":
            if name != partition_name:
                in_names.append(name)
        elif alloc.kind == "ExternalOutput":
            out_names.append(name)
            shape = tuple(alloc.tensor_shape)
            dtype = mb.dt.np(alloc.dtype)
            out_avals.append(jax.core.ShapedArray(shape, dtype))
            zero_outs.append(np.zeros(shape, dtype))
    n_params = len(in_names)
    n_outs = len(out_avals)
    all_in_names = list(in_names) + list(out_names)
    if partition_name is not None:
        all_in_names.append(partition_name)

    def _body(*args):
        operands = list(args)
        if partition_name is not None:
            operands.append(bass2jax.partition_id_tensor())
        outs = bass2jax._bass_exec_p.bind(
            *operands,
            out_avals=tuple(out_avals),
            in_names=tuple(all_in_names),
            out_names=tuple(out_names),
            lowering_input_output_aliases=(),
            sim_require_finite=True,
            sim_require_nnan=True,
            nc=nc,
        )
        return tuple(outs)

    devices = jax.devices()[:NCORES]
    mesh = Mesh(np.asarray(devices), ("core",))
    in_specs = (PartitionSpec("core"),) * (n_params + n_outs)
    out_specs = (PartitionSpec("core"),) * len(out_names)
    donate = tuple(range(n_params, n_params + n_outs))
    sharded = jax.jit(
        shard_map(_body, mesh=mesh, in_specs=in_specs, out_specs=out_specs,
                  check_rep=False),
        donate_argnums=donate, keep_unused=True)

    def run(in_maps):
        concat_in = [
            np.concatenate([np.asarray(in_maps[c][n]) for c in range(NCORES)],
                           axis=0)
            for n in in_names
        ]
        concat_zeros = [
            np.zeros((NCORES * z.shape[0], *z.shape[1:]), z.dtype)
            for z in zero_outs
        ]
        out_arrs = sharded(*concat_in, *concat_zeros)
        return [
            {n: np.asarray(out_arrs[i]).reshape(NCORES, *out_avals[i].shape)[c]
             for i, n in enumerate(out_names)}
            for c in range(NCORES)
        ]

    return run


def kernel(x, W_in, W_rec, W_pred, B_proj, W_dec, b_dec, _nsteps=T, _trace=False):
    x = np.asarray(x, np.float32)
    consts = _host_constants(np.asarray(W_in, np.float32),
                             np.asarray(W_rec, np.float32),
                             np.asarray(W_pred, np.float32),
                             np.asarray(B_proj, np.float32),
                             np.asarray(W_dec, np.float32),
                             np.asarray(b_dec, np.float32))
    key = _nsteps
    if key not in _CACHE:
        nc = _build(_nsteps)
        _CACHE[key] = _make_runner(nc)
    run = _CACHE[key]

    in_maps = []
    for c in range(NCORES):
        xs = x[c * BL:(c + 1) * BL]                      # [8, T, D]
        xT = np.ascontiguousarray(
            xs.transpose(2, 1, 0).reshape(2, 128, T, BL).transpose(1, 0, 2, 3))
        m = dict(consts)
        m["xT"] = xT
        m["nxT"] = np.ascontiguousarray(-xT)
        in_maps.append(m)

    results = run(in_maps)
    y = np.concatenate([results[c]["y"] for c in range(NCORES)], axis=0)
    bd = np.asarray(b_dec, np.float32)
    if np.any(bd):
        y = y + bd[None, None, :]
    return y
